# revision 25
# baseline (speedup 1.0000x reference)
"""Trainium2 Bass kernel for nn_Attention2 (B=4, N=4096, W=1024, H=16, A=64).

Sharding: 8 cores = batch(4) x head-half(2). Each core computes the partial
output sum over its 8 heads for one batch; the host adds the two half-sums.

Math (per batch b, head h):
    c = exp(x@k1 + p1);  e = exp(x@(k2-k3) - p2)   [= diag/(extra*p2e)]
    C[t] = cumsum(c);  den = C + e;  rden = 1/den
    ratio[t] = c[t-1]/c[t]   (via rcpc = exp(-(x@k1 + p1)))
    Shat[t] = ratio[t]*Shat[t-1] + v[t]            (v = x@vw)
    out = Shat*(c*rden) + v*(e*rden);  y = sum_h out @ owT
p1/p2 (sums of 64 near-linear sinusoids) are expanded as cubic polynomials in
n and folded into a small fp32 basis matmul over [1, n, n^2, n^3].

v4 changes vs v3:
  - bf16 data path for x, k-projections, value/output weights, inner, and y
    (halves DMA + SBUF; error ~4e-3 vs the 2e-2 gate). The broadcast path
    (rat/r2/q2 and their PE replications) stays fp32r: the scan multiplies
    long chains of ratios where bf16 error compounds.
  - The 3 per-pair [8->128] broadcast matmuls (rat/r2/q2) are packed into
    ONE concurrent 3-wide group via tile_position row strips 0/32/64: the
    DVE writes rat at partition base 0, r2 at 32, q2 at 64 (cross-quadrant
    32-aligned writes), and the sel constants are replicated per strip.
  - DMA consolidation: descriptor generation costs ~625ns per dma_start
    serialized per ring, so weights load as single batched DMAs and xt
    loads as one DMA per chunk (chunk 0 split per-kb for startup latency).
"""

import numpy as np
import ml_dtypes

import concourse.bacc as bacc
import concourse.mybir as mybir
import concourse.tile as tile
from concourse.bass_utils import run_bass_kernel_spmd

F32 = mybir.dt.float32
F32R = mybir.dt.float32r
BF16 = mybir.dt.bfloat16
AF = mybir.ActivationFunctionType
OP = mybir.AluOpType

B, N, W, H, A, P = 4, 4096, 1024, 16, 64, 64
HL = 8            # heads per core
NPAIR = 4         # head pairs per core
CHUNK = 512
NCHUNK = N // CHUNK          # 8
KB = W // 128                # 8 x-K-blocks
NBLK = CHUNK // 128          # n-blocks per chunk for stage-3

_NC_CACHE = {}


def _build(reps=1, mode="v4"):
    key = ("nc", reps, mode)
    if key in _NC_CACHE:
        return _NC_CACHE[key]
    nc = bacc.Bacc("TRN2")

    xtb = nc.dram_tensor("xtb", [W, N], BF16, kind="ExternalInput")
    basb = nc.dram_tensor("basb", [4, N], F32R, kind="ExternalInput")
    kpack = nc.dram_tensor("kpack", [128, KB, 72], BF16, kind="ExternalInput")
    kp4b = nc.dram_tensor("kp4b", [4, 72], F32R, kind="ExternalInput")
    vwp = nc.dram_tensor("vwp", [128, NPAIR, KB, 128], BF16,
                         kind="ExternalInput")
    owtp = nc.dram_tensor("owtp", [128, NPAIR, W], BF16, kind="ExternalInput")
    selp = nc.dram_tensor("selp", [128, NPAIR, 128], F32R, kind="ExternalInput")
    y = nc.dram_tensor("y", [N, W], BF16, kind="ExternalOutput")

    with tile.TileContext(nc) as tc:
        import os
        _bufs = dict(x.split("=") for x in os.environ.get("KBUFS", "").split(",") if x)
        bf = lambda k, d: int(_bufs.get(k, d))
        with (
            tc.tile_pool(name="const", bufs=1) as const,
            tc.tile_pool(name="xt0p", bufs=1) as xt0p,
            tc.tile_pool(name="xtp", bufs=bf("xt", 2)) as xtp,
            tc.tile_pool(name="rowp", bufs=bf("row", 2)) as rowp,
            tc.tile_pool(name="bigp", bufs=bf("big", 2)) as bigp,
            tc.tile_pool(name="innp", bufs=bf("inn", 8)) as innp,
            tc.tile_pool(name="yp", bufs=bf("yp", 2)) as yp,
            tc.tile_pool(name="rows_ps", bufs=bf("rps", 1), space="PSUM") as rows_ps,
            tc.tile_pool(name="v_ps", bufs=bf("vps", 2), space="PSUM") as v_ps,
            tc.tile_pool(name="bc_ps", bufs=bf("bcps", 1), space="PSUM") as bc_ps,
            tc.tile_pool(name="y_ps", bufs=bf("yps", 2), space="PSUM") as y_ps,
        ):
            # ---- resident weights + chunk-0 x, ring-ordered so the PE
            # starts ASAP: sync ring feeds xt blocks 0-3 then the values
            # weights; scalar ring feeds kp8 then xt blocks 4-7.
            xt0_t = []
            for j in range(4):
                t = xt0p.tile([128, 2, CHUNK], BF16, name=f"xt0_{j}",
                              tag=f"xt0_{j}")
                xt0_t.append(t)
            nc.sync.dma_start(
                out=xt0_t[0],
                in_=xtb[0:256, 0:CHUNK].rearrange("(kb p) n -> p kb n", p=128))
            kp8 = const.tile([128, KB, 72], BF16, name="kp8", tag="kp8")
            nc.scalar.dma_start(out=kp8, in_=kpack[:, :, :])
            nc.scalar.dma_start(
                out=xt0_t[1],
                in_=xtb[256:512, 0:CHUNK].rearrange("(kb p) n -> p kb n",
                                                    p=128))
            nc.sync.dma_start(
                out=xt0_t[2],
                in_=xtb[512:768, 0:CHUNK].rearrange("(kb p) n -> p kb n",
                                                    p=128))
            nc.scalar.dma_start(
                out=xt0_t[3],
                in_=xtb[768:1024, 0:CHUNK].rearrange("(kb p) n -> p kb n",
                                                     p=128))
            kp4 = const.tile([4, 72], F32R, name="kpbas", tag="kpbas")
            nc.sync.dma_start(out=kp4, in_=kp4b[:, :])
            xt_cur = [xt0_t[kb // 2][:, kb % 2, :] for kb in range(KB)]

            def load_xt(ci):
                c0 = ci * CHUNK
                t = xtp.tile([128, KB, CHUNK], BF16, name="xt", tag="xt")
                nc.sync.dma_start(
                    out=t,
                    in_=xtb[:, c0:c0 + CHUNK].rearrange(
                        "(kb p) n -> p kb n", p=128))
                return [t[:, kb, :] for kb in range(KB)]

            # values weights next on sync (needed right after rows), in two
            # halves so pair 0 doesn't wait for the whole 1MB
            vw01 = const.tile([128, 2, KB, 128], BF16, name="vw01", tag="vw01")
            nc.sync.dma_start(out=vw01, in_=vwp[:, 0:2, :, :])
            vw23 = const.tile([128, 2, KB, 128], BF16, name="vw23", tag="vw23")
            nc.sync.dma_start(out=vw23, in_=vwp[:, 2:4, :, :])
            vw_sb = [[(vw01 if p < 2 else vw23)[:, p % 2, kb, :]
                      for kb in range(KB)] for p in range(NPAIR)]

            # all-chunk basis rows (fp32, one small DMA)
            bas_all = const.tile([4, N], F32R, name="bas_all", tag="bas_all")
            nc.scalar.dma_start(out=bas_all, in_=basb[:, :])

            # sel replicated per quantity strip: strip q (partitions 32q..)
            # holds the pair-select weights used by quantity q's broadcast.
            selm = const.tile([128, NPAIR, 128], F32R, name="selm", tag="selm")
            nc.scalar.dma_start(out=selm, in_=selp[:, :, :])

            owt_sb = []

            def load_owt():
                t = const.tile([128, NPAIR, W], BF16, name="owt8", tag="owt8")
                nc.scalar.dma_start(out=t, in_=owtp[:, :, :])
                owt_sb.append(t)

            ones8 = const.tile([HL, CHUNK], F32)
            nc.vector.memset(ones8, 1.0)

            yeng = nc.sync if mode == "ysync" else nc.scalar

            # ---- per-chunk pipeline state ----
            s_prev = [None] * NPAIR     # Shat carry tiles per pair
            c_prev = None               # C-scan carry tile
            c_prev_t = None             # previous chunk's c tile
            pend = None                 # (inner tiles, c0) awaiting stage-3

            total = NCHUNK * reps
            cis = [i % NCHUNK for i in range(total)]

            def emit_back(inner, c0, split=False):
                """Stage 3 for a completed chunk: y matmuls + copies + DMA.

                split=True (final chunk): one y DMA per 128-row block so the
                writeback overlaps the remaining matmuls instead of waiting
                for the whole chunk.
                """
                owt = owt_sb[0]
                y_sb = yp.tile([128, NBLK, W], BF16, tag="y_sb")
                for nb in range(NBLK):
                    for wh in range(2):
                        yps = y_ps.tile([128, 512], F32, tag="y")
                        for p in range(NPAIR):
                            nc.tensor.matmul(
                                yps,
                                lhsT=inner[p][:, nb * 128:(nb + 1) * 128],
                                rhs=owt[:, p, wh * 512:(wh + 1) * 512],
                                start=(p == 0), stop=(p == NPAIR - 1))
                        # final chunk: alternate the PSUM->SBUF copies
                        # between ACT and DVE (DVE is idle at the tail, so
                        # the last copies don't serialize on ACT), and DMA
                        # the very last block per-wh so the final transfer
                        # is half as large.
                        if split and wh == 1:
                            nc.vector.tensor_copy(
                                y_sb[:, nb, wh * 512:(wh + 1) * 512], yps)
                        else:
                            nc.scalar.copy(
                                y_sb[:, nb, wh * 512:(wh + 1) * 512], yps)
                        if split and nb == NBLK - 1:
                            yeng.dma_start(
                                out=y[c0 + nb * 128:c0 + (nb + 1) * 128,
                                      wh * 512:(wh + 1) * 512],
                                in_=y_sb[:, nb, wh * 512:(wh + 1) * 512])
                    if split and nb < NBLK - 1:
                        yeng.dma_start(
                            out=y[c0 + nb * 128:c0 + (nb + 1) * 128, :],
                            in_=y_sb[:, nb, :])
                if not split:
                    yeng.dma_start(
                        out=y[c0:c0 + CHUNK, :].rearrange(
                            "(nb p) w -> p nb w", p=128),
                        in_=y_sb)

            for it in range(total):
                ci = cis[it]
                c0 = ci * CHUNK

                xt = xt_cur
                bas = bas_all[:, c0:c0 + CHUNK]
                if it + 1 < total:
                    xt_next = load_xt(cis[it + 1])
                else:
                    xt_next = None

                # ---- row projections: [72, CHUNK] psum ----
                rows = rows_ps.tile([72, CHUNK], F32, tag="rows")
                for kb in range(KB):
                    nc.tensor.matmul(rows, lhsT=kp8[:, kb, :], rhs=xt[kb],
                                     start=(kb == 0), stop=False)
                nc.tensor.matmul(rows, lhsT=kp4, rhs=bas,
                                 start=False, stop=True)

                # exps: c, rcpc = 1/c, e  [8, CHUNK] each at partitions 0:8
                c_t = rowp.tile([HL, CHUNK], F32R, tag="c_t")
                nc.scalar.activation(c_t, rows[0:8, :], AF.Exp)
                rcpc_t = rowp.tile([HL, CHUNK], F32R, tag="rcpc_t")
                nc.scalar.activation(rcpc_t, rows[32:40, :], AF.Exp)
                e_t = rowp.tile([HL, CHUNK], F32R, tag="e_t")
                nc.scalar.activation(e_t, rows[64:72, :], AF.Exp)

                # broadcast sources live in one [128, CHUNK] mother tile:
                # rat at partitions 0:8 (strip 0), r2 at 32:40 (strip 1),
                # q2 at 64:72 (strip 2) so the three per-pair broadcasts can
                # run concurrently in separate PE row strips.
                bsrc = rowp.tile([128, CHUNK], F32R, tag="bsrc")
                # ratio[t] = c[t-1] * rcpc[t]
                nc.vector.tensor_mul(bsrc[0:8, 1:CHUNK], c_t[:, 0:CHUNK - 1],
                                     rcpc_t[:, 1:CHUNK])
                if c_prev_t is None:
                    # any finite value works: initial Shat state is 0
                    nc.vector.tensor_copy(bsrc[0:8, 0:1], ones8[:, 0:1])
                else:
                    nc.vector.tensor_mul(bsrc[0:8, 0:1],
                                         c_prev_t[:, CHUNK - 1:CHUNK],
                                         rcpc_t[:, 0:1])
                c_prev_t = c_t
                # C = cumsum(c) chunk-chained
                c_ch = rowp.tile([HL, CHUNK], F32, tag="c_ch")
                nc.vector.tensor_tensor_scan(
                    c_ch, data0=ones8, data1=c_t,
                    initial=(0.0 if c_prev is None
                             else c_prev[:, CHUNK - 1:CHUNK]),
                    op0=OP.mult, op1=OP.add)
                c_prev = c_ch
                # den = C + e ; rden = 1/den ; r2 = c*rden ; q2 = e*rden
                den = rowp.tile([HL, CHUNK], F32, tag="den")
                nc.vector.tensor_add(den, c_ch, e_t)
                rden = rowp.tile([HL, CHUNK], F32, tag="rden")
                nc.vector.reciprocal_approx_fast(out=rden, in_=den)
                nc.vector.tensor_mul(bsrc[32:40, :], c_t, rden)
                nc.vector.tensor_mul(bsrc[64:72, :], e_t, rden)

                # ---- values + broadcasts, interleaved on PE ----
                bc_l = [None] * NPAIR

                def emit_values(p):
                    vps = v_ps.tile([128, CHUNK], F32, tag="v")
                    for kb in range(KB):
                        nc.tensor.matmul(vps, lhsT=vw_sb[p][kb], rhs=xt[kb],
                                         start=(kb == 0), stop=(kb == KB - 1))
                    v_sb = bigp.tile([128, CHUNK], F32, tag="v_sb", bufs=4)
                    nc.scalar.copy(v_sb, vps)
                    return v_sb

                def emit_bcast(p):
                    r_rep = bc_ps.tile([128, CHUNK], F32, tag="r_rep")
                    nc.tensor.matmul(r_rep, lhsT=selm[0:8, p, :],
                                     rhs=bsrc[0:8, :],
                                     start=True, stop=True,
                                     tile_position=(0, 0))
                    r2_rep = bc_ps.tile([128, CHUNK], F32, tag="r2_rep")
                    nc.tensor.matmul(r2_rep, lhsT=selm[32:40, p, :],
                                     rhs=bsrc[32:40, :],
                                     start=True, stop=True,
                                     tile_position=(32, 0))
                    q2_rep = bc_ps.tile([128, CHUNK], F32, tag="q2_rep")
                    nc.tensor.matmul(q2_rep, lhsT=selm[64:72, p, :],
                                     rhs=bsrc[64:72, :],
                                     start=True, stop=True,
                                     tile_position=(64, 0))
                    bc_l[p] = (r_rep, r2_rep, q2_rep)

                def emit_backend(p):
                    r_rep, r2_rep, q2_rep = bc_l[p]
                    v_sb = v_sb_l[p]
                    s_sb = bigp.tile([128, CHUNK], F32, tag="s_sb", bufs=8)
                    nc.vector.tensor_tensor_scan(
                        s_sb, data0=r_rep, data1=v_sb,
                        initial=(0.0 if s_prev[p] is None
                                 else s_prev[p][:, CHUNK - 1:CHUNK]),
                        op0=OP.mult, op1=OP.add)
                    s_prev[p] = s_sb
                    t1 = bigp.tile([128, CHUNK], F32, tag="t1")
                    nc.vector.tensor_mul(t1, s_sb, r2_rep)
                    nc.vector.tensor_mul(v_sb, v_sb, q2_rep)
                    inn = innp.tile([128, CHUNK], BF16, name="inner",
                                    tag="inner")
                    nc.gpsimd.tensor_add(inn, t1, v_sb)
                    return inn

                v_sb_l = [None] * NPAIR
                inner = []
                # PE order: v0 v1 v2 bc0 v3 bc1 bc2 bc3
                v_sb_l[0] = emit_values(0)
                v_sb_l[1] = emit_values(1)
                v_sb_l[2] = emit_values(2)
                emit_bcast(0)
                v_sb_l[3] = emit_values(3)
                emit_bcast(1)
                emit_bcast(2)
                emit_bcast(3)
                for p in range(NPAIR):
                    inner.append(emit_backend(p))

                if it == 0:
                    load_owt()

                if mode in ("nolag", "v1ord"):
                    emit_back(inner, c0)
                else:
                    # ---- lagged stage 3 ----
                    if pend is not None:
                        emit_back(*pend)
                    pend = (inner, c0)
                xt_cur = xt_next

            if pend is not None and mode not in ("nolag", "v1ord"):
                emit_back(*pend, split=True)

    nc.finalize()
    _NC_CACHE[key] = nc
    return nc


def _host_prep(x, k1, k2, k3, a1, a2, b1, b2, c, value_weight, output_weight):
    """Build the 8 per-core input maps."""
    x = np.asarray(x, np.float32)
    k1 = np.asarray(k1, np.float32)
    k2 = np.asarray(k2, np.float32)
    k3 = np.asarray(k3, np.float32)
    a1 = np.asarray(a1, np.float64)[..., 0]   # [H, P]
    a2 = np.asarray(a2, np.float64)[..., 0]
    b1 = np.asarray(b1, np.float64)[..., 0]
    b2 = np.asarray(b2, np.float64)[..., 0]
    cc = np.asarray(c, np.float64)[..., 0]
    vw = np.asarray(value_weight, np.float32)   # [H, W, A]
    ow = np.asarray(output_weight, np.float32)  # [H, W, A]
    bft = ml_dtypes.bfloat16

    n = np.linspace(0.0, 1.0, N)
    basis = np.stack([np.ones_like(n), n, n * n, n ** 3]).astype(np.float32)

    def taylor(a, b):
        # coef[k, h] of n^k for sum_p c*sin(a*n+b)
        s, co = np.sin(b), np.cos(b)
        c0 = (cc * s).sum(1)
        c1 = (cc * a * co).sum(1)
        c2 = -(cc * a * a * s).sum(1) / 2.0
        c3 = -(cc * a ** 3 * co).sum(1) / 6.0
        return np.stack([c0, c1, c2, c3])      # [4, H]

    p1c = taylor(a1, b1)
    p2c = taylor(a2, b2)

    xt_by_b = [np.ascontiguousarray(x[b].T).astype(bft) for b in range(B)]

    selp = np.zeros((128, NPAIR, 128), np.float32)
    for q in range(3):                 # quantity strips: rat / r2 / q2
        for p in range(NPAIR):
            selp[32 * q + 2 * p, p, 0:64] = 1.0
            selp[32 * q + 2 * p + 1, p, 64:128] = 1.0

    in_maps = []
    for core in range(8):
        b, half = divmod(core, 2)
        hs = slice(half * HL, (half + 1) * HL)
        kpk = np.zeros((W, 72), np.float32)
        kp4 = np.zeros((4, 72), np.float32)
        # zc -> c = exp(x@k1 + p1)   (row groups 32-aligned: ACT PSUM reads
        # require 32-aligned partition bases)
        kpk[:, 0:8] = k1[hs].T
        kp4[:, 0:8] = p1c[:, hs]
        # -zc -> rcpc = 1/c
        kpk[:, 32:40] = -k1[hs].T
        kp4[:, 32:40] = -p1c[:, hs]
        # ze -> e = exp(x@(k2-k3) - p2)
        kpk[:, 64:72] = (k2[hs] - k3[hs]).T
        kp4[:, 64:72] = -p2c[:, hs]
        # kpack DRAM layout [128, KB, 72]: kpack[p, kb, :] = kpk[kb*128+p, :]
        kpk_kb = np.ascontiguousarray(
            kpk.reshape(KB, 128, 72).transpose(1, 0, 2))

        vwp = np.empty((128, NPAIR, KB, 128), np.float32)
        owtp = np.empty((128, NPAIR, W), np.float32)
        for p in range(NPAIR):
            h0 = half * HL + 2 * p
            vwpair = np.empty((W, 128), np.float32)
            vwpair[:, 0:64] = vw[h0]
            vwpair[:, 64:128] = vw[h0 + 1]
            vwp[:, p, :, :] = vwpair.reshape(KB, 128, 128).transpose(1, 0, 2)
            owtp[0:64, p, :] = ow[h0].T
            owtp[64:128, p, :] = ow[h0 + 1].T

        in_maps.append(dict(
            xtb=xt_by_b[b], basb=basis, kpack=kpk_kb.astype(bft), kp4b=kp4,
            vwp=vwp.astype(bft), owtp=owtp.astype(bft), selp=selp))
    return in_maps


LAST_RESULT = None


def kernel(**inputs) -> np.ndarray:
    global LAST_RESULT
    in_maps = _host_prep(**inputs)
    nc = _build()
    res = None
    for attempt in range(3):
        try:
            res = run_bass_kernel_spmd(nc, in_maps, core_ids=list(range(8)))
            break
        except Exception:
            # transient axon-tunnel / device flakes happen; retry
            if attempt == 2:
                raise
            import time
            time.sleep(5)
    LAST_RESULT = res
    out = np.empty((B, N, W), np.float32)
    for b in range(B):
        out[b] = (res.results[2 * b]["y"].astype(np.float32)
                  + res.results[2 * b + 1]["y"].astype(np.float32))
    return out


# revision 31
# speedup vs baseline: 1.0764x; 1.0764x over previous
"""Trainium2 Bass kernel for nn_Attention2 (B=4, N=4096, W=1024, H=16, A=64).

Sharding: 8 cores = batch(4) x head-half(2). Each core computes the partial
output sum over its 8 heads for one batch; the host adds the two half-sums.

Math (per batch b, head h):
    c = exp(x@k1 + p1);  e = exp(x@(k2-k3) - p2)   [= diag/(extra*p2e)]
    C[t] = cumsum(c);  den = C + e;  rden = 1/den
    ratio[t] = c[t-1]/c[t]   (via rcpc = exp(-(x@k1 + p1)))
    Shat[t] = ratio[t]*Shat[t-1] + v[t]            (v = x@vw)
    out = Shat*(c*rden) + v*(e*rden);  y = sum_h out @ owT
p1/p2 (sums of 64 near-linear sinusoids) are expanded as cubic polynomials in
n and folded into a small fp32 basis matmul over [1, n, n^2, n^3].

v4 changes vs v3:
  - bf16 data path for x, k-projections, value/output weights, inner, and y
    (halves DMA + SBUF; error ~4e-3 vs the 2e-2 gate). The broadcast path
    (rat/r2/q2 and their PE replications) stays fp32r: the scan multiplies
    long chains of ratios where bf16 error compounds.
  - The 3 per-pair [8->128] broadcast matmuls (rat/r2/q2) are packed into
    ONE concurrent 3-wide group via tile_position row strips 0/32/64: the
    DVE writes rat at partition base 0, r2 at 32, q2 at 64 (cross-quadrant
    32-aligned writes), and the sel constants are replicated per strip.
  - DMA consolidation: descriptor generation costs ~625ns per dma_start
    serialized per ring, so weights load as single batched DMAs and xt
    loads as one DMA per chunk (chunk 0 split per-kb for startup latency).
"""

import numpy as np
import ml_dtypes

import concourse.bacc as bacc
import concourse.mybir as mybir
import concourse.tile as tile
from concourse.bass_utils import run_bass_kernel_spmd

F32 = mybir.dt.float32
F32R = mybir.dt.float32r
BF16 = mybir.dt.bfloat16
AF = mybir.ActivationFunctionType
OP = mybir.AluOpType

B, N, W, H, A, P = 4, 4096, 1024, 16, 64, 64
HL = 8            # heads per core
NPAIR = 4         # head pairs per core
CHUNK = 512
NCHUNK = N // CHUNK          # 8
KB = W // 128                # 8 x-K-blocks
NBLK = CHUNK // 128          # n-blocks per chunk for stage-3

_NC_CACHE = {}


def _build(reps=1, mode="v4"):
    key = ("nc", reps, mode)
    if key in _NC_CACHE:
        return _NC_CACHE[key]
    nc = bacc.Bacc("TRN2")

    xtb = nc.dram_tensor("xtb", [W, N], BF16, kind="ExternalInput")
    basb = nc.dram_tensor("basb", [4, N], F32R, kind="ExternalInput")
    kpack = nc.dram_tensor("kpack", [128, KB, 72], BF16, kind="ExternalInput")
    kp4b = nc.dram_tensor("kp4b", [4, 72], F32R, kind="ExternalInput")
    vwp = nc.dram_tensor("vwp", [128, NPAIR, KB, 128], BF16,
                         kind="ExternalInput")
    owtp = nc.dram_tensor("owtp", [128, NPAIR, W], BF16, kind="ExternalInput")
    selp = nc.dram_tensor("selp", [128, NPAIR, 128], F32R, kind="ExternalInput")
    y = nc.dram_tensor("y", [N, W], BF16, kind="ExternalOutput")

    with tile.TileContext(nc) as tc:
        import os
        _bufs = dict(x.split("=") for x in os.environ.get("KBUFS", "").split(",") if x)
        bf = lambda k, d: int(_bufs.get(k, d))
        with (
            tc.tile_pool(name="const", bufs=1) as const,
            tc.tile_pool(name="xt0p", bufs=1) as xt0p,
            tc.tile_pool(name="xtp", bufs=bf("xt", 2)) as xtp,
            tc.tile_pool(name="rowp", bufs=bf("row", 2)) as rowp,
            tc.tile_pool(name="bigp", bufs=bf("big", 2)) as bigp,
            tc.tile_pool(name="innp", bufs=bf("inn", 8)) as innp,
            tc.tile_pool(name="yp", bufs=bf("yp", 2)) as yp,
            tc.tile_pool(name="rows_ps", bufs=bf("rps", 1), space="PSUM") as rows_ps,
            tc.tile_pool(name="v_ps", bufs=bf("vps", 2), space="PSUM") as v_ps,
            tc.tile_pool(name="bc_ps", bufs=bf("bcps", 1), space="PSUM") as bc_ps,
            tc.tile_pool(name="y_ps", bufs=bf("yps", 2), space="PSUM") as y_ps,
        ):
            # ---- resident weights + chunk-0 x, ring-ordered so the PE
            # starts ASAP: sync ring feeds xt blocks 0-3 then the values
            # weights; scalar ring feeds kp8 then xt blocks 4-7.
            xt0_t = []
            for j in range(4):
                t = xt0p.tile([128, 2, CHUNK], BF16, name=f"xt0_{j}",
                              tag=f"xt0_{j}")
                xt0_t.append(t)
            nc.sync.dma_start(
                out=xt0_t[0],
                in_=xtb[0:256, 0:CHUNK].rearrange("(kb p) n -> p kb n", p=128))
            kp8 = const.tile([128, KB, 72], BF16, name="kp8", tag="kp8")
            nc.scalar.dma_start(out=kp8, in_=kpack[:, :, :])
            nc.scalar.dma_start(
                out=xt0_t[1],
                in_=xtb[256:512, 0:CHUNK].rearrange("(kb p) n -> p kb n",
                                                    p=128))
            nc.sync.dma_start(
                out=xt0_t[2],
                in_=xtb[512:768, 0:CHUNK].rearrange("(kb p) n -> p kb n",
                                                    p=128))
            nc.scalar.dma_start(
                out=xt0_t[3],
                in_=xtb[768:1024, 0:CHUNK].rearrange("(kb p) n -> p kb n",
                                                     p=128))
            # basis weights/rows live at partition base 96 (PE row strip 3)
            # so the basis matmul can pack 4-wide with a broadcast triple
            # (strips 0-2), hiding its 512 cycles entirely.
            kp4m = const.tile([100, 72], F32R, name="kpbas", tag="kpbas")
            kp4 = kp4m[96:100, :]
            nc.sync.dma_start(out=kp4, in_=kp4b[:, :])
            xt_cur = [xt0_t[kb // 2][:, kb % 2, :] for kb in range(KB)]

            def load_xt(ci):
                c0 = ci * CHUNK
                t = xtp.tile([128, KB, CHUNK], BF16, name="xt", tag="xt")
                nc.sync.dma_start(
                    out=t,
                    in_=xtb[:, c0:c0 + CHUNK].rearrange(
                        "(kb p) n -> p kb n", p=128))
                return [t[:, kb, :] for kb in range(KB)]

            # values weights next on sync (needed right after rows), in two
            # halves so pair 0 doesn't wait for the whole 1MB
            vw01 = const.tile([128, 2, KB, 128], BF16, name="vw01", tag="vw01")
            nc.sync.dma_start(out=vw01, in_=vwp[:, 0:2, :, :])
            vw23 = const.tile([128, 2, KB, 128], BF16, name="vw23", tag="vw23")
            nc.sync.dma_start(out=vw23, in_=vwp[:, 2:4, :, :])
            vw_sb = [[(vw01 if p < 2 else vw23)[:, p % 2, kb, :]
                      for kb in range(KB)] for p in range(NPAIR)]

            # all-chunk basis rows (fp32, one small DMA), at strip-3 partitions
            basm = const.tile([100, N], F32R, name="bas_all", tag="bas_all")
            nc.scalar.dma_start(out=basm[96:100, :], in_=basb[:, :])

            # sel replicated per quantity strip: strip q (partitions 32q..)
            # holds the pair-select weights used by quantity q's broadcast.
            selm = const.tile([128, NPAIR, 128], F32R, name="selm", tag="selm")
            nc.scalar.dma_start(out=selm, in_=selp[:, :, :])

            owt_sb = []

            def load_owt():
                t = const.tile([128, NPAIR, W], BF16, name="owt8", tag="owt8")
                nc.scalar.dma_start(out=t, in_=owtp[:, :, :])
                owt_sb.append(t)

            ones8 = const.tile([HL, CHUNK], F32)
            nc.vector.memset(ones8, 1.0)

            yeng = nc.sync if mode == "ysync" else nc.scalar

            # ---- per-chunk pipeline state ----
            s_prev = [None] * NPAIR     # Shat carry tiles per pair
            c_prev = None               # C-scan carry tile
            c_prev_t = None             # previous chunk's c tile
            pend = None                 # (inner tiles, c0) awaiting stage-3
            rows_pend = None            # rows psum tile with basis MM done

            def emit_basis(ci):
                """Allocate chunk ci's rows psum tile and run its basis
                matmul (start=True) in PE strip 3; emitted adjacent to a
                broadcast triple (strips 0-2) so the four run concurrently."""
                c0 = ci * CHUNK
                rows_t = rows_ps.tile([72, CHUNK], F32, tag="rows")
                nc.tensor.matmul(rows_t, lhsT=kp4,
                                 rhs=basm[96:100, c0:c0 + CHUNK],
                                 start=True, stop=False,
                                 tile_position=(96, 0))
                return rows_t

            total = NCHUNK * reps
            cis = [i % NCHUNK for i in range(total)]

            def emit_back(inner, c0, split=False):
                """Stage 3 for a completed chunk: y matmuls + copies + DMA.

                split=True (final chunk): one y DMA per 128-row block so the
                writeback overlaps the remaining matmuls instead of waiting
                for the whole chunk.
                """
                owt = owt_sb[0]
                y_sb = yp.tile([128, NBLK, W], BF16, tag="y_sb")
                for nb in range(NBLK):
                    for wh in range(2):
                        yps = y_ps.tile([128, 512], F32, tag="y")
                        for p in range(NPAIR):
                            nc.tensor.matmul(
                                yps,
                                lhsT=inner[p][:, nb * 128:(nb + 1) * 128],
                                rhs=owt[:, p, wh * 512:(wh + 1) * 512],
                                start=(p == 0), stop=(p == NPAIR - 1))
                        # final chunk: alternate the PSUM->SBUF copies
                        # between ACT and DVE (DVE is idle at the tail, so
                        # the last copies don't serialize on ACT), and DMA
                        # the very last block per-wh so the final transfer
                        # is half as large.
                        if split and wh == 1:
                            nc.vector.tensor_copy(
                                y_sb[:, nb, wh * 512:(wh + 1) * 512], yps)
                        else:
                            nc.scalar.copy(
                                y_sb[:, nb, wh * 512:(wh + 1) * 512], yps)
                        if split and nb == NBLK - 1:
                            yeng.dma_start(
                                out=y[c0 + nb * 128:c0 + (nb + 1) * 128,
                                      wh * 512:(wh + 1) * 512],
                                in_=y_sb[:, nb, wh * 512:(wh + 1) * 512])
                    if split and nb < NBLK - 1:
                        yeng.dma_start(
                            out=y[c0 + nb * 128:c0 + (nb + 1) * 128, :],
                            in_=y_sb[:, nb, :])
                if not split:
                    yeng.dma_start(
                        out=y[c0:c0 + CHUNK, :].rearrange(
                            "(nb p) w -> p nb w", p=128),
                        in_=y_sb)

            for it in range(total):
                ci = cis[it]
                c0 = ci * CHUNK

                xt = xt_cur
                if it + 1 < total:
                    xt_next = load_xt(cis[it + 1])
                else:
                    xt_next = None

                # ---- row projections: [72, CHUNK] psum ----
                # chunks 1+: basis already accumulated (start=True) by
                # emit_basis, packed with the previous chunk's last bcast
                # triple. Chunk 0: basis last (stop=True) so the first rows
                # matmuls don't wait on the basis-table DMA.
                if rows_pend is not None:
                    rows = rows_pend
                    for kb in range(KB):
                        nc.tensor.matmul(rows, lhsT=kp8[:, kb, :], rhs=xt[kb],
                                         start=False, stop=(kb == KB - 1))
                else:
                    rows = rows_ps.tile([72, CHUNK], F32, tag="rows")
                    for kb in range(KB):
                        nc.tensor.matmul(rows, lhsT=kp8[:, kb, :], rhs=xt[kb],
                                         start=(kb == 0), stop=False)
                    nc.tensor.matmul(rows, lhsT=kp4,
                                     rhs=basm[96:100, c0:c0 + CHUNK],
                                     start=False, stop=True,
                                     tile_position=(96, 0))

                # exps: c, rcpc = 1/c, e  [8, CHUNK] each at partitions 0:8
                c_t = rowp.tile([HL, CHUNK], F32R, tag="c_t")
                nc.scalar.activation(c_t, rows[0:8, :], AF.Exp)
                rcpc_t = rowp.tile([HL, CHUNK], F32R, tag="rcpc_t")
                nc.scalar.activation(rcpc_t, rows[32:40, :], AF.Exp)
                e_t = rowp.tile([HL, CHUNK], F32R, tag="e_t")
                nc.scalar.activation(e_t, rows[64:72, :], AF.Exp)

                # broadcast sources live in one [128, CHUNK] mother tile:
                # rat at partitions 0:8 (strip 0), r2 at 32:40 (strip 1),
                # q2 at 64:72 (strip 2) so the three per-pair broadcasts can
                # run concurrently in separate PE row strips.
                bsrc = rowp.tile([128, CHUNK], F32R, tag="bsrc")
                # ratio[t] = c[t-1] * rcpc[t]
                nc.vector.tensor_mul(bsrc[0:8, 1:CHUNK], c_t[:, 0:CHUNK - 1],
                                     rcpc_t[:, 1:CHUNK])
                if c_prev_t is None:
                    # any finite value works: initial Shat state is 0
                    nc.vector.tensor_copy(bsrc[0:8, 0:1], ones8[:, 0:1])
                else:
                    nc.vector.tensor_mul(bsrc[0:8, 0:1],
                                         c_prev_t[:, CHUNK - 1:CHUNK],
                                         rcpc_t[:, 0:1])
                c_prev_t = c_t
                # C = cumsum(c) chunk-chained
                c_ch = rowp.tile([HL, CHUNK], F32, tag="c_ch")
                nc.vector.tensor_tensor_scan(
                    c_ch, data0=ones8, data1=c_t,
                    initial=(0.0 if c_prev is None
                             else c_prev[:, CHUNK - 1:CHUNK]),
                    op0=OP.mult, op1=OP.add)
                c_prev = c_ch
                # den = C + e ; rden = 1/den ; r2 = c*rden ; q2 = e*rden
                den = rowp.tile([HL, CHUNK], F32, tag="den")
                nc.vector.tensor_add(den, c_ch, e_t)
                rden = rowp.tile([HL, CHUNK], F32, tag="rden")
                nc.vector.reciprocal_approx_fast(out=rden, in_=den)
                nc.vector.tensor_mul(bsrc[32:40, :], c_t, rden)
                nc.vector.tensor_mul(bsrc[64:72, :], e_t, rden)

                # ---- values + broadcasts, interleaved on PE ----
                bc_l = [None] * NPAIR

                def emit_values(p):
                    vps = v_ps.tile([128, CHUNK], F32, tag="v")
                    for kb in range(KB):
                        nc.tensor.matmul(vps, lhsT=vw_sb[p][kb], rhs=xt[kb],
                                         start=(kb == 0), stop=(kb == KB - 1))
                    v_sb = bigp.tile([128, CHUNK], F32, tag="v_sb", bufs=4)
                    nc.scalar.copy(v_sb, vps)
                    return v_sb

                def emit_bcast(p):
                    r_rep = bc_ps.tile([128, CHUNK], F32, tag="r_rep")
                    nc.tensor.matmul(r_rep, lhsT=selm[0:8, p, :],
                                     rhs=bsrc[0:8, :],
                                     start=True, stop=True,
                                     tile_position=(0, 0))
                    r2_rep = bc_ps.tile([128, CHUNK], F32, tag="r2_rep")
                    nc.tensor.matmul(r2_rep, lhsT=selm[32:40, p, :],
                                     rhs=bsrc[32:40, :],
                                     start=True, stop=True,
                                     tile_position=(32, 0))
                    q2_rep = bc_ps.tile([128, CHUNK], F32, tag="q2_rep")
                    nc.tensor.matmul(q2_rep, lhsT=selm[64:72, p, :],
                                     rhs=bsrc[64:72, :],
                                     start=True, stop=True,
                                     tile_position=(64, 0))
                    bc_l[p] = (r_rep, r2_rep, q2_rep)

                def emit_backend(p):
                    r_rep, r2_rep, q2_rep = bc_l[p]
                    v_sb = v_sb_l[p]
                    s_sb = bigp.tile([128, CHUNK], F32, tag="s_sb", bufs=8)
                    nc.vector.tensor_tensor_scan(
                        s_sb, data0=r_rep, data1=v_sb,
                        initial=(0.0 if s_prev[p] is None
                                 else s_prev[p][:, CHUNK - 1:CHUNK]),
                        op0=OP.mult, op1=OP.add)
                    s_prev[p] = s_sb
                    t1 = bigp.tile([128, CHUNK], F32, tag="t1")
                    nc.vector.tensor_mul(t1, s_sb, r2_rep)
                    nc.vector.tensor_mul(v_sb, v_sb, q2_rep)
                    inn = innp.tile([128, CHUNK], BF16, name="inner",
                                    tag="inner")
                    nc.gpsimd.tensor_add(inn, t1, v_sb)
                    return inn

                v_sb_l = [None] * NPAIR
                inner = []
                # PE order: v0 v1 v2 bc0 v3 bc1 bc2 bc3
                v_sb_l[0] = emit_values(0)
                v_sb_l[1] = emit_values(1)
                v_sb_l[2] = emit_values(2)
                emit_bcast(0)
                v_sb_l[3] = emit_values(3)
                emit_bcast(1)
                emit_bcast(2)
                emit_bcast(3)
                # next chunk's basis MM rides strip 3 of this bcast triple
                rows_pend = emit_basis(cis[it + 1]) if it + 1 < total else None
                for p in range(NPAIR):
                    inner.append(emit_backend(p))

                if it == 0:
                    load_owt()

                if mode in ("nolag", "v1ord"):
                    emit_back(inner, c0)
                else:
                    # ---- lagged stage 3 ----
                    if pend is not None:
                        emit_back(*pend)
                    pend = (inner, c0)
                xt_cur = xt_next

            if pend is not None and mode not in ("nolag", "v1ord"):
                emit_back(*pend, split=True)

    nc.finalize()
    _NC_CACHE[key] = nc
    return nc


def _host_prep(x, k1, k2, k3, a1, a2, b1, b2, c, value_weight, output_weight):
    """Build the 8 per-core input maps."""
    x = np.asarray(x, np.float32)
    k1 = np.asarray(k1, np.float32)
    k2 = np.asarray(k2, np.float32)
    k3 = np.asarray(k3, np.float32)
    a1 = np.asarray(a1, np.float64)[..., 0]   # [H, P]
    a2 = np.asarray(a2, np.float64)[..., 0]
    b1 = np.asarray(b1, np.float64)[..., 0]
    b2 = np.asarray(b2, np.float64)[..., 0]
    cc = np.asarray(c, np.float64)[..., 0]
    vw = np.asarray(value_weight, np.float32)   # [H, W, A]
    ow = np.asarray(output_weight, np.float32)  # [H, W, A]
    bft = ml_dtypes.bfloat16

    n = np.linspace(0.0, 1.0, N)
    basis = np.stack([np.ones_like(n), n, n * n, n ** 3]).astype(np.float32)

    def taylor(a, b):
        # coef[k, h] of n^k for sum_p c*sin(a*n+b)
        s, co = np.sin(b), np.cos(b)
        c0 = (cc * s).sum(1)
        c1 = (cc * a * co).sum(1)
        c2 = -(cc * a * a * s).sum(1) / 2.0
        c3 = -(cc * a ** 3 * co).sum(1) / 6.0
        return np.stack([c0, c1, c2, c3])      # [4, H]

    p1c = taylor(a1, b1)
    p2c = taylor(a2, b2)

    xt_by_b = [np.ascontiguousarray(x[b].T).astype(bft) for b in range(B)]

    selp = np.zeros((128, NPAIR, 128), np.float32)
    for q in range(3):                 # quantity strips: rat / r2 / q2
        for p in range(NPAIR):
            selp[32 * q + 2 * p, p, 0:64] = 1.0
            selp[32 * q + 2 * p + 1, p, 64:128] = 1.0

    in_maps = []
    for core in range(8):
        b, half = divmod(core, 2)
        hs = slice(half * HL, (half + 1) * HL)
        kpk = np.zeros((W, 72), np.float32)
        kp4 = np.zeros((4, 72), np.float32)
        # zc -> c = exp(x@k1 + p1)   (row groups 32-aligned: ACT PSUM reads
        # require 32-aligned partition bases)
        kpk[:, 0:8] = k1[hs].T
        kp4[:, 0:8] = p1c[:, hs]
        # -zc -> rcpc = 1/c
        kpk[:, 32:40] = -k1[hs].T
        kp4[:, 32:40] = -p1c[:, hs]
        # ze -> e = exp(x@(k2-k3) - p2)
        kpk[:, 64:72] = (k2[hs] - k3[hs]).T
        kp4[:, 64:72] = -p2c[:, hs]
        # kpack DRAM layout [128, KB, 72]: kpack[p, kb, :] = kpk[kb*128+p, :]
        kpk_kb = np.ascontiguousarray(
            kpk.reshape(KB, 128, 72).transpose(1, 0, 2))

        vwp = np.empty((128, NPAIR, KB, 128), np.float32)
        owtp = np.empty((128, NPAIR, W), np.float32)
        for p in range(NPAIR):
            h0 = half * HL + 2 * p
            vwpair = np.empty((W, 128), np.float32)
            vwpair[:, 0:64] = vw[h0]
            vwpair[:, 64:128] = vw[h0 + 1]
            vwp[:, p, :, :] = vwpair.reshape(KB, 128, 128).transpose(1, 0, 2)
            owtp[0:64, p, :] = ow[h0].T
            owtp[64:128, p, :] = ow[h0 + 1].T

        in_maps.append(dict(
            xtb=xt_by_b[b], basb=basis, kpack=kpk_kb.astype(bft), kp4b=kp4,
            vwp=vwp.astype(bft), owtp=owtp.astype(bft), selp=selp))
    return in_maps


LAST_RESULT = None


def kernel(**inputs) -> np.ndarray:
    global LAST_RESULT
    in_maps = _host_prep(**inputs)
    nc = _build()
    res = None
    for attempt in range(3):
        try:
            res = run_bass_kernel_spmd(nc, in_maps, core_ids=list(range(8)))
            break
        except Exception:
            # transient axon-tunnel / device flakes happen; retry
            if attempt == 2:
                raise
            import time
            time.sleep(5)
    LAST_RESULT = res
    out = np.empty((B, N, W), np.float32)
    for b in range(B):
        out[b] = (res.results[2 * b]["y"].astype(np.float32)
                  + res.results[2 * b + 1]["y"].astype(np.float32))
    return out
